# revision 19
# baseline (speedup 1.0000x reference)
"""Self-contained Trainium2 Bass kernel for nn_ConfigurableGAT
(3-layer GAT, N=50000, E=800000, 8 NeuronCores). Optimized v4.

vs v2: cross-layer fusion (layer li+1's dense rows are computed inline
right after each block's edge output, so the AllGathers of layer li+1
overlap layer li's edge phase), decreasing-size AllGather groups (only
a tiny final group gates the next edge phase), host-precomputed
dst-major one-hot (Sd) matrices (drops 18 PE transposes + DVE build per
block), bf16 layer-3 gather table with packed al_s, leaky-relu on the
idle Activation engine, PSUM->SBUF copies on the Activation engine.
"""
import numpy as np
from contextlib import ExitStack

import jax
import concourse.bass as bass
import concourse.bacc as bacc
import concourse.mybir as mybir
import concourse.tile as tile
from concourse.library_config import mlp
from concourse import bass2jax
from concourse.bass2jax import _bass_exec_p, install_neuronx_cc_hook
from jax.sharding import Mesh, PartitionSpec
try:
    from jax.experimental.shard_map import shard_map
except ImportError:
    from jax.sharding import shard_map


N_CORES = 8
P = 128


def plan_graph(edge_index, n_nodes=50000, n_cores=N_CORES, k_lo=9, k_hi=9):
    src = np.asarray(edge_index[0], dtype=np.int64)
    dst = np.asarray(edge_index[1], dtype=np.int64)
    E = src.shape[0]

    deg = np.bincount(dst, minlength=n_nodes)   # random edges only
    n_blocks_total = -(-n_nodes // P)
    n_blocks_total = -(-n_blocks_total // n_cores) * n_cores
    n_blocks = n_blocks_total // n_cores
    NP = n_blocks_total * P
    half = NP // 2
    K = k_lo + k_hi

    # ---- stage 1: assign nodes to cores, balancing total in-degree.
    order = np.argsort(-deg, kind="stable")
    core_fill = np.zeros(n_cores, dtype=np.int64)
    core_cnt = np.zeros(n_cores, dtype=np.int64)
    node_core = np.full(n_nodes, -1, dtype=np.int64)
    npc = n_blocks * P
    for nid in order:
        c = np.argmin(np.where(core_cnt < npc, core_fill, np.iinfo(np.int64).max))
        node_core[nid] = c
        core_fill[c] += deg[nid]
        core_cnt[c] += 1

    src_is_lo = node_core[src] < n_cores // 2
    deg_lo = np.bincount(dst[src_is_lo], minlength=n_nodes)
    deg_hi = deg - deg_lo

    # ---- stage 2: per core, pack nodes into blocks with dual caps.
    cap_lo, cap_hi = k_lo * P, k_hi * P
    node_block = np.full(n_nodes, -1, dtype=np.int64)
    for c in range(n_cores):
        nodes = np.where(node_core == c)[0]
        nodes = nodes[np.argsort(-(deg_lo[nodes] + deg_hi[nodes]), kind="stable")]
        bl = np.zeros(n_blocks, dtype=np.int64)
        bh = np.zeros(n_blocks, dtype=np.int64)
        bc = np.zeros(n_blocks, dtype=np.int64)
        for nid in nodes:
            dl, dh = deg_lo[nid], deg_hi[nid]
            ok = (bl + dl <= cap_lo) & (bh + dh <= cap_hi) & (bc < P)
            if not ok.any():
                raise RuntimeError(
                    f"packing failed core {c}: need k_lo/k_hi larger "
                    f"(deg {dl}/{dh}, fills {bl.max()}/{bh.max()})")
            cand = np.where(ok)[0]
            util = np.maximum((bl[cand] + dl) / cap_lo, (bh[cand] + dh) / cap_hi)
            b = cand[np.argmin(util)]
            node_block[nid] = c * n_blocks + b
            bl[b] += dl
            bh[b] += dh
            bc[b] += 1

    # ---- permutation
    perm = np.full(NP, -1, dtype=np.int64)
    inv = np.full(n_nodes, -1, dtype=np.int64)
    fill_cnt = np.zeros(n_blocks_total, dtype=np.int64)
    for nid in range(n_nodes):
        b = node_block[nid]
        slot = b * P + fill_cnt[b]
        fill_cnt[b] += 1
        perm[slot] = nid
        inv[nid] = slot

    # ---- edge layout
    psrc = inv[src]
    pdst = inv[dst]
    pblock = pdst // P
    is_lo = psrc < half
    order_e = np.lexsort((psrc, pdst, ~is_lo, pblock))
    psrc_s = psrc[order_e]
    pdst_s = pdst[order_e]
    pblock_s = pblock[order_e]
    islo_s = is_lo[order_e]

    idx16 = np.zeros((n_blocks_total, K * P), np.int16)
    seg = np.full((n_blocks_total, K * P), -1.0, np.float32)
    starts = np.searchsorted(pblock_s, np.arange(n_blocks_total))
    ends = np.searchsorted(pblock_s, np.arange(n_blocks_total), side="right")
    for b in range(n_blocks_total):
        s, e = starts[b], ends[b]
        lo_mask = islo_s[s:e]
        nlo = int(lo_mask.sum())
        nhi = (e - s) - nlo
        assert nlo <= cap_lo and nhi <= cap_hi, (b, nlo, nhi)
        idx16[b, :nlo] = psrc_s[s:s + nlo]
        seg[b, :nlo] = (pdst_s[s:s + nlo] - b * P)
        idx16[b, cap_lo:cap_lo + nhi] = psrc_s[s + nlo:e] - half
        seg[b, cap_lo:cap_lo + nhi] = (pdst_s[s + nlo:e] - b * P)

    return dict(perm=perm, inv=inv, idx16=idx16, seg=seg, K=K, k_lo=k_lo,
                k_hi=k_hi, n_blocks=n_blocks, NP=NP, half=half,
                n_chunks=n_blocks * K)


def pack_inputs(plan, x, weights, n_cores=N_CORES):
    """Build the per-core input dicts for the bass kernel."""
    import ml_dtypes
    BF = ml_dtypes.bfloat16
    n_blocks, K = plan["n_blocks"], plan["K"]
    NP, perm = plan["NP"], plan["perm"]
    NPC = n_blocks * P
    n_chunks = n_blocks * K
    IN_C = np.asarray(x).shape[1]

    xp = np.zeros((NP, IN_C), np.float32)
    valid = perm >= 0
    xp[valid] = np.asarray(x, np.float32)[perm[valid]]
    xp = xp.astype(BF)

    def wcat(W, a_s, a_d):
        W = np.asarray(W, np.float32)
        IF, F = W.shape
        a_s = np.asarray(a_s, np.float32)
        a_d = np.asarray(a_d, np.float32)
        H, C = a_s.shape
        As = np.zeros((F, H), np.float32)
        Ad = np.zeros((F, H), np.float32)
        for h in range(H):
            As[h * C:(h + 1) * C, h] = a_s[h]
            Ad[h * C:(h + 1) * C, h] = a_d[h]
        Wc = np.concatenate([W, W @ As, W @ Ad], axis=1)
        KT = IF // 128
        return np.ascontiguousarray(
            Wc.reshape(KT, 128, F + 2 * H).transpose(1, 0, 2)).astype(BF)

    w1 = wcat(weights["W1"], weights["a_src1"], weights["a_dst1"])
    w2 = wcat(weights["W2"], weights["a_src2"], weights["a_dst2"])
    w3 = wcat(weights["W3"], weights["a_src3"], weights["a_dst3"])
    b1 = np.broadcast_to(np.asarray(weights["b1"], np.float32), (128, 256)).copy()
    b2 = np.broadcast_to(np.asarray(weights["b2"], np.float32), (128, 256)).copy()
    b3 = np.broadcast_to(np.asarray(weights["b3"], np.float32), (128, 64)).copy()

    # idx wrapped for dma_gather, one wrap per GATHER GROUP (must mirror
    # gather_groups): elem i of group -> [16k + i%16, g0*8 + i//16]
    k_lo, k_hi = plan["k_lo"], plan["k_hi"]
    gb_sz = plan.get("gather_batch", 1)
    groups = []
    for base, cnt in ((0, k_lo), (k_lo, k_hi)):
        j = 0
        while j < cnt:
            n = min(gb_sz, cnt - j)
            groups.append((base + j, n))
            j += n
    idx_in = np.zeros((n_cores, 128, n_chunks * 8), np.int16)
    seg_in = np.zeros((n_cores, 128, n_chunks), np.float32)
    sd_in = np.zeros((n_cores, 128, n_chunks * 128), BF)
    iota128 = np.arange(128, dtype=np.float32)[:, None]
    for c in range(n_cores):
        for blk in range(n_blocks):
            gb = c * n_blocks + blk
            for (j0, ng) in groups:
                flat = plan["idx16"][gb, j0 * P:(j0 + ng) * P]
                wrap = flat.reshape(ng * 8, 16).T     # [16, ng*8]
                g0 = blk * K + j0
                idx_in[c, :, g0 * 8:g0 * 8 + ng * 8] = np.tile(wrap, (8, 1))
            for j in range(K):
                g = blk * K + j
                seg_in[c, :, g] = plan["seg"][gb, j * P:(j + 1) * P]
        # dst-major one-hots: sd[d, (b j e)] = 1 iff seg[b, j*128+e] == d
        segs = plan["seg"][c * n_blocks:(c + 1) * n_blocks].reshape(1, -1)
        sd_in[c] = (segs == iota128).astype(BF)
    iotakf = np.broadcast_to(
        np.tile(np.arange(128, dtype=np.float32), K), (128, K * 128)).copy()
    ins = []
    for c in range(n_cores):
        ins.append({
            "x": xp[c * NPC:(c + 1) * NPC].copy(),
            "idx": idx_in[c],
            "seg": seg_in[c],
            "sd": sd_in[c],
            "wcat1": w1, "wcat2": w2, "wcat3": w3,
            "bias1": b1, "bias2": b2, "bias3": b3,
            "iden": np.eye(128, dtype=np.float32),
            "iotakf": iotakf,
            "asrc1": _asrc_flat(weights["a_src1"], BF),
            "asrc2": _asrc_flat(weights["a_src2"], BF),
        })
    return ins


def _asrc_flat(a, dt):
    flat = np.asarray(a, np.float32).reshape(-1)
    return np.broadcast_to(flat, (128, flat.shape[0])).astype(dt).copy()


def unpack_output(plan, outs, n_nodes=50000):
    perm, NP = plan["perm"], plan["NP"]
    full = np.concatenate([o["out"] for o in outs], axis=0)
    assert full.shape[0] == NP
    valid = perm >= 0
    res = np.zeros((n_nodes, full.shape[1]), np.float32)
    res[perm[valid]] = full[valid]
    return res


F32 = mybir.dt.float32
BF16 = mybir.dt.bfloat16
AF = mybir.ActivationFunctionType
OP = mybir.AluOpType


def gather_groups(k_lo, k_hi, gb):
    groups = []
    for base, cnt in ((0, k_lo), (k_lo, k_hi)):
        j = 0
        while j < cnt:
            n = min(gb, cnt - j)
            groups.append((base + j, n, base == 0))
            j += n
    return groups


SKIP_AG = False      # timing-probe only: drop the AllGathers
SKIP_GATHER = False  # timing-probe only: drop the dma_gathers

# AllGather group sizes (blocks per group), decreasing so the final
# group -- the only one that gates the next edge phase -- is tiny.
AG_SPLITS_49 = [49]


def build_gat_v4(n_blocks: int, k_lo: int, k_hi: int, n_cores: int = 8,
                 in_feat: int = 256, gather_batch: int = 9):
    """Input tensor names (per core):
      x      [NPC, in_feat] bf16
      idx    [128, n_chunks*8] int16  (wrapped+replicated dma_gather indices)
      seg    [128, n_chunks] f32      (local dst 0..127, -1 pad)
      sd     [128, n_chunks*128] bf16 (host-built dst-major one-hots)
      wcat1  [128, in_feat//128, 272] bf16; wcat2 [128,2,272]; wcat3 [128,2,66]
      asrc1, asrc2 [128, 256] bf16 replicated a_src flat
      bias1, bias2 [128, 256] f32; bias3 [128, 64] f32
      iden [128,128] f32; iotakf [128, K*128] f32
    Output: out [NPC, 64] f32
    """
    P = 128
    K = k_lo + k_hi
    NPC = n_blocks * P
    NP = NPC * n_cores
    half = NP // 2
    n_chunks = n_blocks * K
    if n_blocks == sum(AG_SPLITS_49):
        splits = AG_SPLITS_49
    else:
        splits = [n_blocks]
    bounds = set(np.cumsum(splits))
    assert sorted(bounds)[-1] == n_blocks

    nc = bacc.Bacc("TRN2", target_bir_lowering=False, debug=False,
                   num_devices=n_cores)

    x_in = nc.dram_tensor("x", [NPC, in_feat], BF16, kind="ExternalInput")
    idx_in = nc.dram_tensor("idx", [128, n_chunks * 8], mybir.dt.int16,
                            kind="ExternalInput")
    seg_in = nc.dram_tensor("seg", [128, n_chunks], F32, kind="ExternalInput")
    sd_in = nc.dram_tensor("sd", [128, n_chunks * 128], BF16,
                           kind="ExternalInput")
    kt1 = in_feat // 128
    wcat_in = [
        nc.dram_tensor("wcat1", [128, kt1, 272], BF16, kind="ExternalInput"),
        nc.dram_tensor("wcat2", [128, 2, 272], BF16, kind="ExternalInput"),
        nc.dram_tensor("wcat3", [128, 2, 66], BF16, kind="ExternalInput"),
    ]
    asrc_in = [
        nc.dram_tensor("asrc1", [128, 256], BF16, kind="ExternalInput"),
        nc.dram_tensor("asrc2", [128, 256], BF16, kind="ExternalInput"),
    ]
    bias_in = [
        nc.dram_tensor("bias1", [128, 256], F32, kind="ExternalInput"),
        nc.dram_tensor("bias2", [128, 256], F32, kind="ExternalInput"),
        nc.dram_tensor("bias3", [128, 64], F32, kind="ExternalInput"),
    ]
    iden_in = nc.dram_tensor("iden", [128, 128], F32, kind="ExternalInput")
    iotakf_in = nc.dram_tensor("iotakf", [128, K * 128], F32,
                               kind="ExternalInput")
    out_ext = nc.dram_tensor("out", [NPC, 64], F32, kind="ExternalOutput")
    import os
    DBG = bool(os.environ.get("GAT_DBG"))

    # (F, H, C, in_feat, table dtype, table width)
    LAY = [(256, 8, 32, in_feat, BF16, 256), (256, 8, 32, 256, BF16, 256),
           (64, 1, 64, 256, BF16, 128)]
    # columns of the dense output stored into the gather table (layer 3
    # packs al_s at column 64)
    TCOLS = [256, 256, 65]

    # Per-AllGather-group tensors: a collective reading/writing a tensor
    # serializes (at whole-tensor granularity) against any concurrent
    # access, so each group gets its own input (T_locg) and output slab
    # (T_compg). Layer 0 uses a single whole-table group.
    gbounds = [0] + sorted(bounds)
    n_groups = len(gbounds) - 1
    T_locg = [[nc.dram_tensor(f"Tloc{i}g{g}",
                              [(gbounds[g + 1] - gbounds[g]) * P, LAY[i][5]],
                              LAY[i][4])
               for g in range(n_groups)] if i else
              [nc.dram_tensor(f"Tloc0g0", [NPC, LAY[i][5]], LAY[i][4])]
              for i in range(3)]
    T_full = [nc.dram_tensor(f"Tfull{i}", [NP, LAY[i][5]], LAY[i][4],
                             addr_space="Shared")
              for i in range(3)]
    T_compg = [[nc.dram_tensor(f"Tcomp{i}g{g}",
                               [(gbounds[g + 1] - gbounds[g]) * P * n_cores,
                                LAY[i][5]],
                               LAY[i][4], addr_space="Shared")
                for g in range(n_groups)]
               for i in range(3)]

    def tloc_of(li, b):
        """(tensor, row offset) holding block b of layer li's local table."""
        if li == 0:
            return T_locg[0][0], b * P
        g = next(gi for gi in range(n_groups) if gbounds[gi + 1] > b)
        return T_locg[li][g], (b - gbounds[g]) * P
    act_d = [None,
             nc.dram_tensor("act2", [NPC, 256], BF16),
             nc.dram_tensor("act3", [NPC, 256], BF16)]

    replica_groups = [list(range(n_cores))]

    with ExitStack() as ctx:
        tc = ctx.enter_context(tile.TileContext(nc))
        const = ctx.enter_context(tc.tile_pool(name="const", bufs=1))
        sb = ctx.enter_context(tc.tile_pool(name="sb", bufs=2))
        sb3 = ctx.enter_context(tc.tile_pool(name="sb3", bufs=3))
        ps = ctx.enter_context(tc.tile_pool(name="ps", bufs=2, space="PSUM"))

        nc.gpsimd.load_library(mlp)

        ident = const.tile([128, 128], F32)
        nc.sync.dma_start(out=ident[:], in_=iden_in[:])
        ident_b = const.tile([128, 128], BF16)
        nc.vector.tensor_copy(out=ident_b[:], in_=ident[:])
        iotakf = const.tile([128, K * 128], F32)
        nc.sync.dma_start(out=iotakf[:], in_=iotakf_in[:])
        idx_t = const.tile([128, n_chunks * 8], mybir.dt.int16)
        nc.sync.dma_start(out=idx_t[:], in_=idx_in[:])
        seg_t = const.tile([128, n_chunks], F32)
        nc.sync.dma_start(out=seg_t[:], in_=seg_in[:])

        # per-layer constants, loaded once
        wc_t, bia_t, asr_t, ald_t = [], [], [], []
        for li in range(3):
            F, H, C, IF, TDl, TW = LAY[li]
            KT = IF // 128
            C2 = F + 2 * H
            wc = const.tile([128, KT, C2], BF16, tag=f"wc{li}")
            nc.sync.dma_start(out=wc[:], in_=wcat_in[li][:])
            wc_t.append(wc)
            bia = const.tile([128, F], F32, tag=f"bias{li}")
            nc.sync.dma_start(out=bia[:], in_=bias_in[li][:])
            bia_t.append(bia)
            if li < 2:
                asr = const.tile([128, F], TDl, tag=f"asrc{li}")
                nc.sync.dma_start(out=asr[:], in_=asrc_in[li][:])
                asr_t.append(asr)
            else:
                asr_t.append(None)
            ald_t.append(const.tile([128, n_blocks, H], TDl, tag=f"ald{li}",
                                    name=f"aldt{li}"))

        Tf3 = [T_full[i][:].rearrange("(r n) c -> r n c", r=n_cores)
               for i in range(3)]

        def dense_block(li, rb, act_ap):
            """T_loc[li] rows of block rb + al_d column, from act_ap."""
            F, H, C, IF, TDl, TW = LAY[li]
            KT = IF // 128
            C2 = F + 2 * H
            tc_cols = TCOLS[li]
            att = sb3.tile([128, KT, 128], BF16, tag="att")
            for kt in range(KT):
                q = nc.scalar if kt % 2 == 0 else nc.sync
                q.dma_start_transpose(
                    out=att[:, kt, :],
                    in_=act_ap[rb * P:(rb + 1) * P, kt * 128:(kt + 1) * 128])
            pd = ps.tile([128, 272], F32, tag="pdense")
            for kt in range(KT):
                nc.tensor.matmul(pd[:, :C2], lhsT=att[:, kt, :],
                                 rhs=wc_t[li][:, kt, :],
                                 start=(kt == 0), stop=(kt == KT - 1))
            trow = sb3.tile([128, tc_cols], TDl, tag=f"trow{li}")
            nc.scalar.copy(out=trow[:], in_=pd[:, :tc_cols])
            tl, off = tloc_of(li, rb)
            nc.sync.dma_start(out=tl[off:off + P, :tc_cols], in_=trow[:])
            nc.vector.tensor_copy(out=ald_t[li][:, rb, :],
                                  in_=pd[:, F + H:C2])

        # Un-interleave copies are deferred (DELAY group boundaries) so the
        # copy's wait-on-AllGather is already satisfied at issue time --
        # otherwise the in-order HWDGE queues head-of-line block the edge
        # loop's DMAs behind a ~100us collective wait.
        pending = []
        AG_COPY_DELAY = 2

        def flush_one():
            li, g, r0, r1 = pending.pop(0)
            slab3 = T_compg[li][g][:].rearrange("(r n) c -> r n c", r=n_cores)
            rm = min(r0 + max((r1 - r0) // 2, P), r1)
            nc.sync.dma_start(out=Tf3[li][:, r0:rm, :],
                              in_=slab3[:, :rm - r0, :])
            if rm < r1:
                nc.scalar.dma_start(out=Tf3[li][:, rm:r1, :],
                                    in_=slab3[:, rm - r0:, :])

        def flush_all():
            while pending:
                flush_one()

        def ag_group(li, g, r0b, r1b):
            """AllGather T_locg[li][g] (blocks [r0b, r1b)) into T_full[li]."""
            if SKIP_AG:
                return
            r0, r1 = r0b * P, r1b * P
            if r0 == 0 and r1 == NPC:
                # whole table in one group: the [r, n, c] slab IS the natural
                # core-major table order -- gather straight into T_full
                nc.gpsimd.collective_compute(
                    "AllGather", OP.bypass,
                    replica_groups=replica_groups,
                    ins=[T_locg[li][0][:, :]],
                    outs=[Tf3[li]],
                )
                return
            slab3 = T_compg[li][g][:].rearrange("(r n) c -> r n c", r=n_cores)
            nc.gpsimd.collective_compute(
                "AllGather", OP.bypass,
                replica_groups=replica_groups,
                ins=[T_locg[li][g][:, :]],
                outs=[slab3],
            )
            pending.append((li, g, r0, r1))
            if len(pending) > AG_COPY_DELAY:
                flush_one()

        # ---------------- phase A: layer-1 dense from x, one big AllGather
        for rb in range(n_blocks):
            dense_block(0, rb, x_in)
        ag_group(0, 0, 0, n_blocks)

        # ---------------- layers: edge phase + inline next-layer dense
        for li in range(3):
            F, H, C, IF, TDl, TW = LAY[li]
            C2 = F + 2 * H
            KH = (K + 1) * H
            Tf = T_full[li]

            if li < 2:
                asr_rep = sb.tile([128, K + 1, F], TDl, tag="asrep")
                nc.vector.tensor_copy(
                    out=asr_rep[:],
                    in_=asr_t[li][:, None, :].to_broadcast([128, K + 1, F]))

            prev = 0
            gidx = 0
            for b in range(n_blocks):
                G_all = sb.tile([128, K + 1, TW], TDl, tag="G")
                for gi, (j0, ng, is_lo) in enumerate(
                        [] if SKIP_GATHER else
                        gather_groups(k_lo, k_hi, gather_batch)):
                    g0 = b * K + j0
                    src_view = Tf[:half, :] if is_lo else Tf[half:, :]
                    nc.gpsimd.dma_gather(
                        out_ap=G_all[:, j0:j0 + ng, :],
                        in_ap=src_view,
                        idxs_ap=idx_t[:, g0 * 8:g0 * 8 + ng * 8],
                        num_idxs=ng * 128,
                        num_idxs_reg=ng * 128,
                        elem_size=TW,
                        queue_num=0,
                    )
                # self rows as chunk K
                tl, off = tloc_of(li, b)
                nc.sync.dma_start(out=G_all[:, K, :TCOLS[li]],
                                  in_=tl[off:off + P, :TCOLS[li]])

                # S (edge-major one-hot) for all K chunks in one op
                S_all = sb.tile([128, K, 128], TDl, tag="S")
                nc.vector.tensor_tensor(
                    out=S_all[:],
                    in0=iotakf[:].rearrange("p (k d) -> p k d", k=K),
                    in1=seg_t[:, b * K:(b + 1) * K, None].to_broadcast(
                        [128, K, 128]),
                    op=OP.is_equal)

                # Sd (dst-major one-hot) from host; scatter al_d to edges
                sd_t = sb.tile([128, K, 128], BF16, tag="sd")
                nc.scalar.dma_start(
                    out=sd_t[:],
                    in_=sd_in[:, b * K * 128:(b + 1) * K * 128])
                ald_ps = ps.tile([128, K * H], F32, tag="aldps")
                for j in range(K):
                    nc.tensor.matmul(
                        ald_ps[:, j * H:(j + 1) * H],
                        lhsT=sd_t[:, j, :], rhs=ald_t[li][:, b, :],
                        start=True, stop=True)

                ald_all = sb3.tile([128, KH], F32, tag="alda")
                nc.scalar.copy(out=ald_all[:, :K * H], in_=ald_ps[:])
                nc.vector.tensor_copy(out=ald_all[:, K * H:],
                                      in_=ald_t[li][:, b, :])

                # al_s for all chunks (incl self)
                als_all = sb3.tile([128, KH], F32, tag="alsa")
                if li < 2:
                    tmp = sb3.tile([128, (K + 1) * F], TDl, tag="alstmp")
                    nc.vector.tensor_tensor(
                        out=tmp[:],
                        in0=G_all[:].rearrange("p k f -> p (k f)"),
                        in1=asr_rep[:].rearrange("p k f -> p (k f)"),
                        op=OP.mult)
                    nc.vector.tensor_reduce(
                        out=als_all[:],
                        in_=tmp[:].rearrange("p (g c) -> p g c", c=C),
                        axis=mybir.AxisListType.X, op=OP.add)
                else:
                    # al_s is packed at table column 64
                    nc.vector.tensor_copy(out=als_all[:],
                                          in_=G_all[:, :, 64])

                # logits -> leaky relu (Act engine) -> exp (Act engine)
                lg = sb3.tile([128, KH], F32, tag="lg")
                nc.vector.tensor_tensor(out=lg[:], in0=ald_all[:],
                                        in1=als_all[:], op=OP.add)
                lt = sb3.tile([128, KH], F32, tag="lt")
                nc.vector.scalar_tensor_tensor(
                    out=lt[:], in0=lg[:], scalar=0.2, in1=lg[:],
                    op0=OP.mult, op1=OP.max)
                exb = sb3.tile([128, KH], TDl, tag="exb")
                nc.scalar.activation(out=exb[:], in_=lt[:], func=AF.Exp)

                # rhs = [G*ex | ex]
                rhs = sb.tile([128, K + 1, H, C + 1], TDl, tag="rhs")
                exv = exb[:].rearrange("p (k h) -> p k h", h=H)
                nc.vector.tensor_tensor(
                    out=rhs[:, :, :, :C],
                    in0=G_all[:, :, :F].rearrange("p k (h c) -> p k h c", h=H),
                    in1=exv[:, :, :, None].to_broadcast([128, K + 1, H, C]),
                    op=OP.mult)
                nc.vector.tensor_copy(out=rhs[:, :, :, C], in_=exv)

                # aggregation
                pa = ps.tile([128, H * (C + 1)], F32, tag="pagg")
                for j in range(K + 1):
                    lhs_agg = S_all[:, j, :] if j < K else ident_b[:]
                    nc.tensor.matmul(
                        pa[:], lhsT=lhs_agg,
                        rhs=rhs[:, j].rearrange("p h c -> p (h c)"),
                        start=(j == 0), stop=(j == K))

                # normalize + bias (+ELU)
                pa3 = pa[:].rearrange("p (h c) -> p h c", h=H)
                dn = sb3.tile([128, H], F32, tag="dn")
                nc.vector.tensor_scalar_add(dn[:], pa3[:, :, C], 1e-30)
                rc = sb3.tile([128, H], F32, tag="rc")
                nc.vector.reciprocal(rc[:], dn[:])
                ob = sb3.tile([128, F], F32, tag="ob")
                ob3 = ob[:].rearrange("p (h c) -> p h c", h=H)
                nc.vector.tensor_tensor(out=ob3, in0=pa3[:, :, :C],
                                        in1=rc[:, :, None].to_broadcast(
                                            [128, H, C]),
                                        op=OP.mult)
                nc.vector.tensor_tensor(out=ob[:], in0=ob[:], in1=bia_t[li][:],
                                        op=OP.add)
                if li < 2:
                    # elu(x) = max(x,0) - 1 + exp(min(x,0))
                    mn = sb3.tile([128, F], F32, tag="mn")
                    nc.vector.tensor_scalar_min(mn[:], ob[:], 0.0)
                    em = sb3.tile([128, F], F32, tag="em")
                    nc.scalar.activation(out=em[:], in_=mn[:], func=AF.Exp)
                    acb = sb3.tile([128, F], F32, tag="acb")
                    nc.vector.tensor_scalar(
                        out=acb[:], in0=ob[:], scalar1=0.0, scalar2=-1.0,
                        op0=OP.max, op1=OP.add)
                    actb = sb3.tile([128, F], BF16, tag="actb")
                    nc.vector.tensor_tensor(out=actb[:], in0=acb[:], in1=em[:],
                                            op=OP.add)
                    nc.sync.dma_start(
                        out=act_d[li + 1][b * P:(b + 1) * P, :], in_=actb[:])
                else:
                    nc.sync.dma_start(out=out_ext[b * P:(b + 1) * P, :],
                                      in_=ob[:])
            if li < 2:
                # next-layer dense + one whole-table AllGather straight into
                # the gather table
                for rb in range(n_blocks):
                    dense_block(li + 1, rb, act_d[li + 1])
                ag_group(li + 1, 0, 0, n_blocks)
            flush_all()

        if DBG:
            dbgT = [nc.dram_tensor(f"dbgT{i}", [NP, LAY[i][5]], LAY[i][4],
                                   kind="ExternalOutput") for i in range(3)]
            dbgA = [nc.dram_tensor(f"dbgA{i}", [NPC, 256], BF16,
                                   kind="ExternalOutput") for i in range(2)]
            for i in range(3):
                nc.sync.dma_start(out=dbgT[i][:], in_=T_full[i][:])
            for i in range(2):
                nc.scalar.dma_start(out=dbgA[i][:], in_=act_d[i + 1][:])

    nc.compile()
    return nc


class SpmdRunner:
    def __init__(self, nc, n_cores: int):
        install_neuronx_cc_hook()
        self.nc = nc
        self.n_cores = n_cores
        in_names, out_names, out_avals, zero_outs = [], [], [], []
        partition_name = nc.partition_id_tensor.name if nc.partition_id_tensor else None
        for alloc in nc.m.functions[0].allocations:
            if not isinstance(alloc, mybir.MemoryLocationSet):
                continue
            name = alloc.memorylocations[0].name
            if alloc.kind == "ExternalInput":
                if name != partition_name:
                    in_names.append(name)
            elif alloc.kind == "ExternalOutput":
                out_names.append(name)
                shape = tuple(alloc.tensor_shape)
                dtype = mybir.dt.np(alloc.dtype)
                out_avals.append(jax.core.ShapedArray(shape, dtype))
                zero_outs.append(np.zeros(shape, dtype))
        self.in_names, self.out_names = in_names, out_names
        self.out_avals, self.zero_outs = out_avals, zero_outs
        n_params = len(in_names)
        n_outs = len(out_avals)
        all_in_names = list(in_names) + list(out_names)
        if partition_name is not None:
            all_in_names.append(partition_name)

        def _body(*args):
            operands = list(args)
            if partition_name is not None:
                operands.append(bass2jax.partition_id_tensor())
            outs = _bass_exec_p.bind(
                *operands,
                out_avals=tuple(out_avals),
                in_names=tuple(all_in_names),
                out_names=tuple(out_names),
                lowering_input_output_aliases=(),
                sim_require_finite=True,
                sim_require_nnan=True,
                nc=nc,
            )
            return tuple(outs)

        devices = jax.devices()[:n_cores]
        self.mesh = Mesh(np.asarray(devices), ("core",))
        in_specs = (PartitionSpec("core"),) * (n_params + n_outs)
        out_specs = (PartitionSpec("core"),) * n_outs
        self.fn = jax.jit(
            shard_map(_body, mesh=self.mesh, in_specs=in_specs,
                      out_specs=out_specs, check_rep=False),
            keep_unused=True,
        )
        self.dev_in = None

    def set_inputs(self, in_maps):
        concat_in = [
            np.concatenate([np.asarray(in_maps[c][name]) for c in range(self.n_cores)], axis=0)
            for name in self.in_names
        ]
        concat_zeros = [
            np.zeros((self.n_cores * z.shape[0], *z.shape[1:]), z.dtype)
            for z in self.zero_outs
        ]
        sharding = jax.sharding.NamedSharding(self.mesh, PartitionSpec("core"))
        self.dev_in = [jax.device_put(a, sharding) for a in concat_in + concat_zeros]

    def __call__(self):
        outs = self.fn(*self.dev_in)
        jax.block_until_ready(outs)
        return outs

    def results(self, outs):
        per_core = []
        for c in range(self.n_cores):
            d = {}
            for i, name in enumerate(self.out_names):
                full = np.asarray(outs[i])
                sh = self.out_avals[i].shape
                d[name] = full.reshape(self.n_cores, *sh)[c]
            per_core.append(d)
        return per_core


# ======================================================================
# kernel() entry point
# ======================================================================

_CACHE = {}
_RUNNER = {}

N_NODES = 50000
K_TRY = [(9, 9), (10, 10), (12, 12)]
GATHER_BATCH = 3


def _get_compiled(n_blocks, k_lo, k_hi):
    key = (n_blocks, k_lo, k_hi)
    if key not in _CACHE:
        _CACHE[key] = build_gat_v4(n_blocks=n_blocks, k_lo=k_lo, k_hi=k_hi,
                                   gather_batch=min(GATHER_BATCH, k_lo))
    return _CACHE[key]


def _plan_any(edge_index, n_nodes):
    last = None
    for k_lo, k_hi in K_TRY:
        try:
            plan = plan_graph(edge_index, n_nodes=n_nodes, k_lo=k_lo, k_hi=k_hi)
            plan["gather_batch"] = min(GATHER_BATCH, k_lo)
            return plan
        except RuntimeError as e:
            last = e
    raise last


def kernel(**inputs):
    x = np.asarray(inputs["x"])
    edge_index = np.asarray(inputs["edge_index"])
    n_nodes = x.shape[0]
    weights = {k: np.asarray(v) for k, v in inputs.items()
               if k not in ("x", "edge_index")}

    plan = _plan_any(edge_index, n_nodes)
    ins = pack_inputs(plan, x, weights)

    key = (plan["n_blocks"], plan["k_lo"], plan["k_hi"])
    nc = _get_compiled(*key)
    if key not in _RUNNER:
        _RUNNER[key] = SpmdRunner(nc, 8)
    r = _RUNNER[key]
    r.set_inputs(ins)
    outs = r()
    res = unpack_output(plan, r.results(outs), n_nodes=n_nodes)
    return res.astype(np.float32)
